# revision 7
# baseline (speedup 1.0000x reference)
"""FCOS loss kernel for Trainium2, data-parallel over batch across 8 NeuronCores.

Math (per batch b), with q = sigmoid(-x), p = 1-q, sp = softplus(x) = -ln(q):
  focal sum = -0.75*S1 - A + B
    S1 = sum_all p^2*ln(q)          (fneg = 0.75*p^2*sp = -0.75*p^2*ln q)
    A  = sum_pos 0.25*q^2*ln(p)     (fpos = 0.25*q^2*(-ln p))
    B  = sum_pos 0.75*p^2*ln(q)     (fneg at the selected logit)
  cnt sum  = sum -(ln(qc) + x*t)*mask,  mask = cnt_t > -1
  reg sum  = sum (1 - giou)*mask
  losses   = mean_b(sum_b / max(npos_b, 1))

Layout (per core = 2 batches):
  cls logits in class-major grouped tiles: partition p = cl*16 + chunk,
  class = g*8 + cl, position = chunk*F_l + f.  Levels padded so each level's
  hw is a multiple of 128.  The target-class logit is selected via one-hot
  (tensor_scalar is_equal against per-partition class ids) * x, then a PE
  matmul with a 0/1 lhsT reduces over the 8 class rows per group; psum
  accumulates over the 10 groups.
"""

import sys
import numpy as np

sys.path.insert(0, "/opt/trn_rl_repo")

import ml_dtypes

BF16 = ml_dtypes.bfloat16

# ---- problem geometry (hardcoded) ----
B, C, S = 16, 80, 17064
NCORES = 8
LEVELS = [(100, 128), (50, 64), (25, 32), (13, 16), (7, 8)]
HW_REAL = [h * w for h, w in LEVELS]          # 12800, 3200, 800, 208, 56
HW_PAD = [12800, 3200, 896, 256, 128]         # multiples of 128
F_L = [hw // 16 for hw in HW_PAD]             # 800, 200, 56, 16, 8
W_L = [hw // 128 for hw in HW_PAD]            # 100, 25, 7, 2, 1
FSUM = sum(F_L)                               # 1080
CC = sum(W_L)                                 # 135
CCP = CC + 1                                  # 136 (even, incl. pad col)
NG, K, G = 10, 16, 8
XCOLS = 2 * NG * FSUM                         # 21600
TCOLS = 2 * FSUM                              # 2160
S0 = np.cumsum([0] + HW_REAL).tolist()        # level offsets in S

PAD_X = -20.0

_cache = {}


# ---------------- host-side data prep ----------------

def _prep_core(ci, inp):
    """Build per-core device arrays for batches (2*ci, 2*ci+1)."""
    batches = (2 * ci, 2 * ci + 1)

    xcls = np.full((128, XCOLS), PAD_X, dtype=np.float32)
    t8 = np.zeros((128, TCOLS), dtype=np.float32)
    tclsc = np.zeros((2, 128, CCP), dtype=np.float32)
    cntx = np.zeros((2, 128, CCP), dtype=np.float32)
    cntt = np.full((2, 128, CCP), -1.0, dtype=np.float32)
    reg8 = np.ones((2, 8, 128, CCP), dtype=np.float32)

    for bi, b in enumerate(batches):
        tcls_b = np.asarray(inp["cls_targets"][b, :, 0], dtype=np.float32)
        cntt_b = np.asarray(inp["cnt_targets"][b, :, 0], dtype=np.float32)
        regt_b = np.asarray(inp["reg_targets"][b], dtype=np.float32)  # [S,4]
        for l in range(5):
            hwr, hwp, F, W = HW_REAL[l], HW_PAD[l], F_L[l], W_L[l]
            coff = sum(W_L[:l])
            loff = sum(F_L[:l])

            # cls logits [80, hwp], padded with PAD_X
            x = np.asarray(inp[f"cls_p{l}"][b], dtype=np.float32).reshape(C, hwr)
            xp = np.full((C, hwp), PAD_X, dtype=np.float32)
            xp[:, :hwr] = x
            for g in range(NG):
                col = (bi * NG + g) * FSUM + loff
                blk = xp[g * G:(g + 1) * G].reshape(G * K, F)  # [128, F]
                xcls[:, col:col + F] = blk

            # cls targets for this level, padded with 0
            t = np.zeros(hwp, dtype=np.float32)
            t[:hwr] = tcls_b[S0[l]:S0[l + 1]]
            t8[:, bi * FSUM + loff: bi * FSUM + loff + F] = \
                np.tile(t.reshape(K, F), (G, 1))
            tclsc[bi, :, coff:coff + W] = t.reshape(128, W)

            # centerness pred/targets
            cx = np.zeros(hwp, dtype=np.float32)
            cx[:hwr] = np.asarray(inp[f"cnt_p{l}"][b], dtype=np.float32).reshape(hwr)
            cntx[bi, :, coff:coff + W] = cx.reshape(128, W)
            ct = np.full(hwp, -1.0, dtype=np.float32)
            ct[:hwr] = cntt_b[S0[l]:S0[l + 1]]
            cntt[bi, :, coff:coff + W] = ct.reshape(128, W)

            # reg pred channels (l,t,r,b) and target channels
            rp = np.asarray(inp[f"reg_p{l}"][b], dtype=np.float32).reshape(4, hwr)
            rt = regt_b[S0[l]:S0[l + 1]].T  # [4, hwr]
            for ch in range(4):
                rpp = np.ones(hwp, dtype=np.float32)
                rpp[:hwr] = rp[ch]
                reg8[bi, ch, :, coff:coff + W] = rpp.reshape(128, W)
                rtp = np.ones(hwp, dtype=np.float32)
                rtp[:hwr] = rt[ch]
                reg8[bi, 4 + ch, :, coff:coff + W] = rtp.reshape(128, W)

    # constants
    p = np.arange(128)
    cbf = np.zeros((128, 17), dtype=np.float32)
    for m in range(16):
        cbf[:, m] = (p % K == m).astype(np.float32)               # lhsT16
    cbf[:, 16] = 1.0                                              # ones
    cf32 = np.ones((128, 1 + NG), dtype=np.float32)
    for g in range(NG):
        cf32[:, 1 + g] = (g * G + p // K + 1).astype(np.float32)  # class ids

    return {
        "xcls": xcls.astype(BF16),
        "t8": t8.astype(BF16),
        "tclsc": tclsc,
        "cntx": cntx,
        "cntt": cntt,
        "reg8": reg8,
        "cbf": cbf.astype(BF16),
        "cf32": cf32,
    }


# ---------------- device kernel ----------------

NQ = 4                 # ACT slabs per batch
NH = NG * FSUM // NQ   # 2700 columns per slab
# ACC cols: 0-1 S1_b (partition 0 only), 2-3 Araw_b, 4-5 Braw_b,
#           6-7 npos_b, 8-9 cntneg_b, 10-11 reg_b
NACC = 12


def build_kernel():
    import concourse.bass as bass  # noqa: F401
    import concourse.tile as tile
    from concourse import bacc, mybir
    from concourse.alu_op_type import AluOpType as op

    f32 = mybir.dt.float32
    bf16 = mybir.dt.bfloat16
    AF = mybir.ActivationFunctionType
    AX = mybir.AxisListType

    nc = bacc.Bacc("TRN2", target_bir_lowering=False, debug=False,
                   enable_asserts=False, num_devices=NCORES)

    d_xcls = nc.dram_tensor("xcls", [128, XCOLS], bf16, kind="ExternalInput").ap()
    d_t8 = nc.dram_tensor("t8", [128, TCOLS], bf16, kind="ExternalInput").ap()
    d_tclsc = nc.dram_tensor("tclsc", [2, 128, CCP], f32, kind="ExternalInput").ap()
    d_cntx = nc.dram_tensor("cntx", [2, 128, CCP], f32, kind="ExternalInput").ap()
    d_cntt = nc.dram_tensor("cntt", [2, 128, CCP], f32, kind="ExternalInput").ap()
    d_reg8 = nc.dram_tensor("reg8", [2, 8, 128, CCP], f32, kind="ExternalInput").ap()
    d_cbf = nc.dram_tensor("cbf", [128, 17], bf16, kind="ExternalInput").ap()
    d_cf32 = nc.dram_tensor("cf32", [128, 1 + NG], f32, kind="ExternalInput").ap()
    d_out = nc.dram_tensor("out", [1, 8], f32, kind="ExternalOutput").ap()

    with tile.TileContext(nc) as tc:
        with (
            tc.tile_pool(name="persist", bufs=1) as persist,
            tc.tile_pool(name="sel", bufs=2) as selp,
            tc.tile_pool(name="act", bufs=2) as actp,
            tc.tile_pool(name="cpt", bufs=1) as cpt,
            tc.tile_pool(name="psum", bufs=1, space="PSUM") as psum,
            tc.tile_pool(name="psumS", bufs=1, space="PSUM") as psumS,
        ):
            # ---- persistent buffers ----
            XALL = persist.tile([128, XCOLS], bf16)   # x, then q, then ln(q)
            PPALL = persist.tile([128, XCOLS], bf16)  # p^2
            T8ALL = persist.tile([128, TCOLS], bf16)
            CBF = persist.tile([128, 17], bf16)
            CF32 = persist.tile([128, 1 + NG], f32)
            ACC = persist.tile([128, NACC], f32)
            XSELC = [persist.tile([128, CCP], f32, tag=f"xselc{b}",
                                  name=f"xselc{b}") for b in range(2)]
            QSEL = [persist.tile([128, CCP], f32, tag=f"qsel{b}",
                                 name=f"qsel{b}") for b in range(2)]
            PSELC = [persist.tile([128, CCP], f32, tag=f"pselc{b}",
                                  name=f"pselc{b}") for b in range(2)]
            QC = [persist.tile([128, CCP], f32, tag=f"qc{b}",
                               name=f"qc{b}") for b in range(2)]
            CNTX = [persist.tile([128, CCP], f32, tag=f"cntxt{b}",
                                 name=f"cntxt{b}") for b in range(2)]

            nc.sync.dma_start(CBF[:], d_cbf)
            nc.sync.dma_start(CF32[:], d_cf32)
            nc.sync.dma_start(T8ALL[:], d_t8)
            for b in range(2):
                nc.vector.memset(XSELC[b][:], 0.0)
            nc.vector.memset(ACC[:], 0.0)

            for b in range(2):
                for g in range(NG):
                    c0 = (b * NG + g) * FSUM
                    nc.sync.dma_start(XALL[:, c0:c0 + FSUM],
                                      d_xcls[:, c0:c0 + FSUM])
            for b in range(2):
                nc.sync.dma_start(CNTX[b][:], d_cntx[b])

            lhsT = CBF[:, 0:16]

            # ---- phase 1a: one-hot select target-class logits (DVE+PE) ----
            for b in range(2):
                t8b = T8ALL[:, b * FSUM:(b + 1) * FSUM]
                chunks = []  # (level, start_in_level, width, psum_tile)
                for l in range(5):
                    F = F_L[l]
                    f0 = 0
                    while f0 < F:
                        wchunk = min(512, F - f0)
                        pt = psum.tile([16, wchunk], f32, tag=f"xsel_l{l}_{f0}",
                                       name=f"xselp_l{l}_{f0}")
                        chunks.append((l, f0, wchunk, pt))
                        f0 += wchunk
                for g in range(NG):
                    c0 = (b * NG + g) * FSUM
                    oh = selp.tile([128, FSUM], bf16, tag="oh")
                    nc.vector.tensor_scalar(
                        out=oh[:], in0=t8b, scalar1=CF32[:, 1 + g:2 + g],
                        scalar2=None, op0=op.is_equal)
                    ohx = selp.tile([128, FSUM], bf16, tag="ohx")
                    nc.vector.tensor_tensor(
                        out=ohx[:], in0=oh[:], in1=XALL[:, c0:c0 + FSUM],
                        op=op.mult)
                    for (l, f0, wchunk, pt) in chunks:
                        loff = sum(F_L[:l])
                        nc.tensor.matmul(
                            pt[:], lhsT, ohx[:, loff + f0:loff + f0 + wchunk],
                            start=(g == 0), stop=(g == NG - 1))
                # psum chunks -> [16, FSUM] staging
                stage = selp.tile([16, FSUM], f32, tag="stage")
                for (l, f0, wchunk, pt) in chunks:
                    loff = sum(F_L[:l])
                    nc.vector.tensor_copy(
                        stage[:, loff + f0:loff + f0 + wchunk], pt[:])
                # reshape-DMA staging -> compact [128, W_l] blocks
                for l in range(5):
                    loff = sum(F_L[:l])
                    coff = sum(W_L[:l])
                    F, W = F_L[l], W_L[l]
                    src = stage[:, loff:loff + F].rearrange(
                        "m (a w) -> m a w", w=W)
                    nc.sync.dma_start(XSELC[b][:, coff:coff + W], src)

            # ---- phase 1b: sigmoid pass (ACT): q = sigmoid(-x) in place ----
            for b in range(2):
                for h in range(NQ):
                    c0 = b * NG * FSUM + h * NH
                    qsl = XALL[:, c0:c0 + NH]
                    nc.scalar.activation(qsl, qsl, AF.Sigmoid, scale=-1.0)
                    # p = 1 - q  (bf16 4x), pp = p*p
                    pt = actp.tile([128, NH], bf16, tag="pt")
                    nc.vector.tensor_scalar(
                        out=pt[:], in0=qsl, scalar1=1.0, scalar2=-1.0,
                        op0=op.subtract, op1=op.mult)
                    nc.vector.tensor_tensor(
                        out=PPALL[:, c0:c0 + NH], in0=pt[:], in1=pt[:],
                        op=op.mult)
            for b in range(2):
                nc.scalar.activation(QSEL[b][:], XSELC[b][:], AF.Sigmoid,
                                     scale=-1.0)
                nc.scalar.activation(QC[b][:], CNTX[b][:], AF.Sigmoid,
                                     scale=-1.0)

            # compact: p = 1-q and squares (DVE, f32) before ln overwrites
            Q2 = []
            P2C = []
            for b in range(2):
                nc.vector.tensor_scalar(
                    out=PSELC[b][:], in0=QSEL[b][:], scalar1=1.0, scalar2=-1.0,
                    op0=op.subtract, op1=op.mult)
                q2 = cpt.tile([128, CCP], f32, tag=f"q2_{b}", name=f"q2_{b}")
                nc.vector.tensor_tensor(out=q2[:], in0=QSEL[b][:],
                                        in1=QSEL[b][:], op=op.mult)
                Q2.append(q2)
                p2c = cpt.tile([128, CCP], f32, tag=f"p2c_{b}", name=f"p2c_{b}")
                nc.vector.tensor_tensor(out=p2c[:], in0=PSELC[b][:],
                                        in1=PSELC[b][:], op=op.mult)
                P2C.append(p2c)

            # ---- phase 2: ln pass (ACT) + S1 via TT + PE colsum ----
            ones_bf = CBF[:, 16:17]
            for b in range(2):
                ps1 = psumS.tile([1, 512], f32, tag="ps1",
                                 name=f"ps1_{b}")
                nmm = 0
                for h in range(NQ):
                    c0 = b * NG * FSUM + h * NH
                    qsl = XALL[:, c0:c0 + NH]
                    nc.scalar.activation(qsl, qsl, AF.Ln)  # ln(q) in place
                    scr = actp.tile([128, NH], bf16, tag="pt")
                    nc.vector.tensor_tensor(
                        out=scr[:], in0=PPALL[:, c0:c0 + NH], in1=qsl,
                        op=op.mult)
                    j0 = 0
                    while j0 < NH:
                        wch = min(512, NH - j0)
                        nc.tensor.matmul(ps1[:, 0:wch], ones_bf,
                                         scr[:, j0:j0 + wch],
                                         start=(nmm == 0),
                                         stop=(h == NQ - 1 and j0 + wch >= NH))
                        nmm += 1
                        j0 += wch
                nc.vector.tensor_reduce(ACC[0:1, b:b + 1], ps1[:],
                                        axis=AX.X, op=op.add)
            for b in range(2):
                # ln(q_sel), ln(p_sel), ln(q_cnt) in place
                nc.scalar.activation(QSEL[b][:], QSEL[b][:], AF.Ln)
                nc.scalar.activation(PSELC[b][:], PSELC[b][:], AF.Ln)
                nc.scalar.activation(QC[b][:], QC[b][:], AF.Ln)

            # ---- compact per-batch: corr, cnt, npos, reg ----
            def tt(out_, a, b_, o):
                nc.vector.tensor_tensor(out=out_, in0=a, in1=b_, op=o)

            for b in range(2):
                def scratch(tag):
                    return cpt.tile([128, CCP], f32, tag=tag, name=tag)[:]

                tcl = scratch("tcl")
                nc.sync.dma_start(tcl, d_tclsc[b])
                mpos = scratch("mpos")
                nc.vector.tensor_scalar(out=mpos, in0=tcl, scalar1=0.5,
                                        scalar2=None, op0=op.is_gt)
                # t1 = q^2*ln(p); t2 = p^2*ln(q)
                t1 = scratch("t1")
                tt(t1, Q2[b][:], PSELC[b][:], op.mult)
                t2 = scratch("t2")
                tt(t2, P2C[b][:], QSEL[b][:], op.mult)
                scr1 = scratch("scr1")
                tt(scr1, t1, mpos, op.mult)
                nc.vector.tensor_reduce(ACC[:, 2 + b:3 + b], scr1,
                                        axis=AX.X, op=op.add)
                scr2 = scratch("scr2")
                tt(scr2, t2, mpos, op.mult)
                nc.vector.tensor_reduce(ACC[:, 4 + b:5 + b], scr2,
                                        axis=AX.X, op=op.add)

                # centerness + npos
                ctt = scratch("ctt")
                nc.sync.dma_start(ctt, d_cntt[b])
                cm = scratch("cm")
                nc.vector.tensor_scalar(out=cm, in0=ctt, scalar1=-0.5,
                                        scalar2=None, op0=op.is_gt)
                nc.vector.tensor_reduce(ACC[:, 6 + b:7 + b], cm,
                                        axis=AX.X, op=op.add)
                xt = scratch("xt")
                tt(xt, CNTX[b][:], ctt, op.mult)
                summ = scratch("summ")
                tt(summ, QC[b][:], xt, op.add)  # ln(qc) + x*t = -bce
                scr4 = scratch("scr4")
                tt(scr4, summ, cm, op.mult)
                nc.vector.tensor_reduce(ACC[:, 8 + b:9 + b], scr4,
                                        axis=AX.X, op=op.add)

                # GIoU
                rch = []
                for ch in range(8):
                    t = cpt.tile([128, CCP], f32, tag=f"rch{ch}",
                                 name=f"rch{ch}")[:]
                    nc.sync.dma_start(t, d_reg8[b, ch])
                    rch.append(t)
                lp, tp, rp, bp, lt_, tt_, rt, bt = rch

                lm, tm, rm, bm = (scratch("lm"), scratch("tm"),
                                  scratch("rm"), scratch("bm"))
                tt(lm, lp, lt_, op.min)
                tt(tm, tp, tt_, op.min)
                tt(rm, rp, rt, op.min)
                tt(bm, bp, bt, op.min)
                wmin, hmin = scratch("wmin"), scratch("hmin")
                tt(wmin, lm, rm, op.add)
                nc.vector.tensor_scalar(out=wmin, in0=wmin, scalar1=0.0,
                                        scalar2=None, op0=op.max)
                tt(hmin, tm, bm, op.add)
                nc.vector.tensor_scalar(out=hmin, in0=hmin, scalar1=0.0,
                                        scalar2=None, op0=op.max)
                ov = scratch("ov")
                tt(ov, wmin, hmin, op.mult)
                w1, h1, a1 = scratch("w1"), scratch("h1"), scratch("a1")
                tt(w1, lp, rp, op.add)
                tt(h1, tp, bp, op.add)
                tt(a1, w1, h1, op.mult)
                w2, h2, a2 = scratch("w2"), scratch("h2"), scratch("a2")
                tt(w2, lt_, rt, op.add)
                tt(h2, tt_, bt, op.add)
                tt(a2, w2, h2, op.mult)
                un = scratch("un")
                tt(un, a1, a2, op.add)
                tt(un, un, ov, op.subtract)
                runion = scratch("runion")
                nc.vector.reciprocal(runion, un)
                iou = scratch("iou")
                tt(iou, ov, runion, op.mult)
                lM, tM, rM, bM = (scratch("lM"), scratch("tM"),
                                  scratch("rM"), scratch("bM"))
                tt(lM, lp, lt_, op.max)
                tt(tM, tp, tt_, op.max)
                tt(rM, rp, rt, op.max)
                tt(bM, bp, bt, op.max)
                wmax, hmax = scratch("wmax"), scratch("hmax")
                tt(wmax, lM, rM, op.add)
                nc.vector.tensor_scalar(out=wmax, in0=wmax, scalar1=0.0,
                                        scalar2=None, op0=op.max)
                tt(hmax, tM, bM, op.add)
                nc.vector.tensor_scalar(out=hmax, in0=hmax, scalar1=0.0,
                                        scalar2=None, op0=op.max)
                ga = scratch("ga")
                tt(ga, wmax, hmax, op.mult)
                gc = scratch("gc")
                nc.vector.tensor_scalar(out=gc, in0=ga, scalar1=1e-10,
                                        scalar2=None, op0=op.max)
                rg = scratch("rg")
                nc.vector.reciprocal(rg, gc)
                gmu = scratch("gmu")
                tt(gmu, ga, un, op.subtract)
                tt(gmu, gmu, rg, op.mult)
                giou = scratch("giou")
                tt(giou, iou, gmu, op.subtract)
                lossel = scratch("lossel")
                nc.vector.tensor_scalar(out=lossel, in0=giou, scalar1=1.0,
                                        scalar2=-1.0, op0=op.subtract,
                                        op1=op.mult)
                scr5 = scratch("scr5")
                tt(scr5, lossel, cm, op.mult)
                nc.vector.tensor_reduce(ACC[:, 10 + b:11 + b], scr5,
                                        axis=AX.X, op=op.add)

            # ---- final reduction over partitions + scalar math ----
            fin = psumS.tile([1, NACC], f32, tag="fin", name="fin")
            nc.tensor.matmul(fin[:], CF32[:, 0:1], ACC[:], start=True,
                             stop=True)
            R = persist.tile([1, NACC], f32)
            nc.vector.tensor_copy(R[:], fin[:])
            OUTT = persist.tile([1, 8], f32)
            # cls_sum = -0.75*S1 - 0.25*Araw + 0.75*Braw
            clsum = persist.tile([1, 2], f32)
            nc.vector.tensor_scalar(out=clsum[:], in0=R[:, 0:2], scalar1=-0.75,
                                    scalar2=None, op0=op.mult)
            ta = persist.tile([1, 2], f32)
            nc.vector.tensor_scalar(out=ta[:], in0=R[:, 2:4], scalar1=0.25,
                                    scalar2=None, op0=op.mult)
            nc.vector.tensor_tensor(out=clsum[:], in0=clsum[:], in1=ta[:],
                                    op=op.subtract)
            tb = persist.tile([1, 2], f32)
            nc.vector.tensor_scalar(out=tb[:], in0=R[:, 4:6], scalar1=0.75,
                                    scalar2=None, op0=op.mult)
            nc.vector.tensor_tensor(out=clsum[:], in0=clsum[:], in1=tb[:],
                                    op=op.add)
            npc = persist.tile([1, 2], f32)
            nc.vector.tensor_scalar(out=npc[:], in0=R[:, 6:8], scalar1=1.0,
                                    scalar2=None, op0=op.max)
            rnp = persist.tile([1, 2], f32)
            nc.vector.reciprocal(rnp[:], npc[:])
            nc.vector.tensor_tensor(out=OUTT[:, 0:2], in0=clsum[:], in1=rnp[:],
                                    op=op.mult)
            cntn = persist.tile([1, 2], f32)
            nc.vector.tensor_scalar(out=cntn[:], in0=R[:, 8:10], scalar1=-1.0,
                                    scalar2=None, op0=op.mult)
            nc.vector.tensor_tensor(out=OUTT[:, 2:4], in0=cntn[:],
                                    in1=rnp[:], op=op.mult)
            nc.vector.tensor_tensor(out=OUTT[:, 4:6], in0=R[:, 10:12],
                                    in1=rnp[:], op=op.mult)
            nc.vector.tensor_copy(OUTT[:, 6:8], npc[:])
            nc.sync.dma_start(d_out, OUTT[:])

    nc.compile()
    return nc


def get_nc():
    if "nc" not in _cache:
        _cache["nc"] = build_kernel()
    return _cache["nc"]


def _combine(outs):
    """outs: [8, 8] per-core device outputs -> final (4,) loss vector."""
    cls_b = outs[:, 0:2].reshape(-1)
    cnt_b = outs[:, 2:4].reshape(-1)
    reg_b = outs[:, 4:6].reshape(-1)
    cls_loss = float(np.mean(cls_b))
    cnt_loss = float(np.mean(cnt_b))
    reg_loss = float(np.mean(reg_b))
    total = cls_loss + cnt_loss + reg_loss
    return np.array([cls_loss, cnt_loss, reg_loss, total], dtype=np.float32)


def kernel(**inputs):
    from concourse import bass_utils

    nc = get_nc()
    in_maps = [_prep_core(ci, inputs) for ci in range(NCORES)]
    res = bass_utils.run_bass_kernel_spmd(
        nc, in_maps, core_ids=list(range(NCORES)))
    _cache["last_results"] = res
    outs = np.stack([r["out"][0] for r in res.results])  # [8, 8]
    return _combine(outs)
